# revision 14
# baseline (speedup 1.0000x reference)
"""Trainium2 Bass kernel for nn_CustomConv2d: 3x3 conv, B=16, Cin=Cout=128, H=W=64.

Strategy (v2, bf16):
  - Data-parallel over batch: 8 NeuronCores x 2 images each; the (128,128,9)
    weight is replicated (host pre-transposes it to [cin, k, cout] so tap k is
    a contiguous [cin, cout] stationary-operand slice).
  - Per image the feature map lives in SBUF as a 66x66 zero-padded plane
    (host-prepadded, so every DMA is fully contiguous):
      row r in [-1,64], col c in [-1,64] at offset (r+1)*66 + (c+1).
  - Conv = 9 accumulating PE matmuls per 8-row output block (contraction over
    Cin=128 on the partition dim).  Tap (dy,dx) reads the 2D window
    [[66,8],[1,64]] at offset (y0+dy)*66 + dx; the padding zeros make every
    tap exact, so there is no edge fixup of any kind.
  - bf16 operands (RNE-rounded on host): same 1 cycle/row PE speed as fp32r,
    but half the DMA bytes and a ~2x faster LDWEIGHTS, which is what gates
    the back-to-back matmul cadence on the Tensor queue.
  - Minimal 2-matmul PE warm-up (bf16 data lands early enough that real
    matmuls carry the HAM clock ramp themselves), few DMA triggers (6 in /
    10 out, stores batched 2 blocks each) to cut per-instruction queue cost
    and the Tile framework's per-event teardown chatter at kernel exit.
"""

import numpy as np
import ml_dtypes

import concourse.bass as bass  # noqa: F401  (registers bass types)
import concourse.tile as tile
import concourse.mybir as mybir
from concourse import bacc, bass_utils

F32 = mybir.dt.float32
BF16 = mybir.dt.bfloat16

B, CIN, COUT, KK, H, W = 16, 128, 128, 3, 64, 64
NCORES = 8
BPC = B // NCORES  # batches per core
HW = H * W         # 4096
PW = W + 2         # padded row length (66)
PH = H + 2         # padded rows (66)
XLEN = PH * PW     # 4356
ROWBLK = 8         # output rows per PSUM block (8*64=512 = one fp32 PSUM bank)
NBLK = H // ROWBLK

TRACE = False      # set True to capture an NTFF profile (fills LAST_EXEC_NS)
LAST_EXEC_NS = None

_CACHE = {}


def _build():
    nc = bacc.Bacc("TRN2", target_bir_lowering=False, debug=False, num_devices=NCORES)
    x_d = nc.dram_tensor("x", [BPC, CIN, XLEN], BF16, kind="ExternalInput").ap()
    w_d = nc.dram_tensor("w", [CIN, KK * KK * COUT], BF16, kind="ExternalInput").ap()
    o_d = nc.dram_tensor("o", [BPC, COUT, HW], F32, kind="ExternalOutput").ap()

    with tile.TileContext(nc) as tc:
        with (
            tc.tile_pool(name="wt", bufs=1) as wtp,
            tc.tile_pool(name="xin", bufs=2) as xp,
            tc.tile_pool(name="ps", bufs=4, space="PSUM") as pp,
            tc.tile_pool(name="ot", bufs=3) as op,
            tc.tile_pool(name="warm", bufs=1) as wmp,
            tc.tile_pool(name="warmps", bufs=1, space="PSUM") as wpp,
        ):
            # Short PE warm-up: just enough array activity to cover the first
            # input DMA's latency; the real matmuls continue the HAM clock
            # ramp (full speed arrives ~3.4us after sustained activity starts
            # either way).
            wz = wmp.tile([CIN, 4 * COUT], BF16)
            nc.vector.memset(wz[:], 0.0)
            wps = wpp.tile([COUT, 4 * COUT], F32)
            NWARM = 3
            for i in range(NWARM):
                # one accumulation chain + distinct stationary slices so no
                # compiler pass drops any of them
                nc.tensor.matmul(
                    wps[:],
                    wz[:, i * COUT : (i + 1) * COUT],
                    wz[:],
                    start=(i == 0),
                    stop=(i == NWARM - 1),
                )

            # Critical first bytes split across BOTH HWDGE rings (the DMA
            # clock is still ramping here, ~180 GB/s per ring): sync ring
            # streams the first input rows while the scalar ring streams the
            # weights.  Strict consumption order within each ring; all the
            # non-critical bulk follows the critical pieces.
            # Block yb touches padded rows [8*yb, 8*yb+9].
            wt = wtp.tile([CIN, KK * KK * COUT], BF16)
            xins = []
            for lb in range(BPC):
                xin = xp.tile([CIN, XLEN], BF16, tag="xin")
                xins.append(xin)
            nc.sync.dma_start(xins[0][:, : PW * 10], x_d[0][:, : PW * 10])
            nc.scalar.dma_start(wt[:, : 3 * COUT], w_d[:, : 3 * COUT])
            nc.scalar.dma_start(wt[:, 3 * COUT :], w_d[:, 3 * COUT :])
            for r0, r1 in [(10, 38), (38, PH)]:
                nc.sync.dma_start(
                    xins[0][:, PW * r0 : PW * r1], x_d[0][:, PW * r0 : PW * r1]
                )
            for r0, r1 in [(0, 34), (34, PH)]:
                nc.sync.dma_start(
                    xins[1][:, PW * r0 : PW * r1], x_d[1][:, PW * r0 : PW * r1]
                )

            for lb in range(BPC):
                xrf = xins[lb][:].rearrange("p (r c) -> p r c", c=PW)  # [128,66,66]
                ot2 = None
                for yb in range(NBLK):
                    y0 = yb * ROWBLK
                    ps = pp.tile([COUT, ROWBLK * W], F32)
                    for t in range(KK * KK):
                        dy, dx = divmod(t, KK)
                        nc.tensor.matmul(
                            ps[:],
                            wt[:, t * COUT : (t + 1) * COUT],
                            xrf[:, y0 + dy : y0 + dy + ROWBLK, dx : dx + W],
                            start=(t == 0),
                            stop=(t == KK * KK - 1),
                        )
                    BLK = ROWBLK * W  # 512
                    if lb == BPC - 1 and yb == NBLK - 1:
                        # final block: halves copied by two engines (vector +
                        # scalar/ACT) and stored via two rings so the copy/
                        # store drain finishes sooner.  GPSIMD cannot read
                        # PSUM, so the second copy uses the ACT engine.
                        oths = []
                        for h_, cp in enumerate(
                            (nc.vector.tensor_copy, nc.scalar.copy)
                        ):
                            sl = slice(h_ * BLK // 2, (h_ + 1) * BLK // 2)
                            oth = op.tile([COUT, BLK // 2], F32, tag=f"oth{h_}")
                            cp(oth[:], ps[:, sl])
                            oths.append(oth)
                        for h_, deng in enumerate((nc.scalar, nc.sync)):
                            nc_lo = W * y0 + h_ * BLK // 2
                            deng.dma_start(
                                o_d[lb][:, nc_lo : nc_lo + BLK // 2], oths[h_][:]
                            )
                    elif lb == BPC - 1 and yb == NBLK - 2:
                        # penultimate block: its own small store
                        oth = op.tile([COUT, BLK], F32)
                        nc.vector.tensor_copy(oth[:], ps[:])
                        nc.scalar.dma_start(o_d[lb][:, W * y0 : W * y0 + BLK], oth[:])
                    else:
                        # batch 2 row-blocks per store
                        pair = yb % 2
                        if pair == 0:
                            ot2 = op.tile([COUT, 2 * BLK], F32)
                        nc.vector.tensor_copy(ot2[:, pair * BLK : (pair + 1) * BLK], ps[:])
                        if pair == 1:
                            nc.scalar.dma_start(
                                o_d[lb][:, W * y0 - BLK : W * y0 + BLK], ot2[:]
                            )
    nc.compile()
    return nc


def _get_nc():
    if "nc" not in _CACHE:
        _CACHE["nc"] = _build()
    return _CACHE["nc"]


def kernel(x, weights):
    """x: [16,128,64,64] f32; weights: [128,128,9] f32 -> [2048,64,64] f32."""
    global LAST_EXEC_NS
    x = np.asarray(x, dtype=np.float32)
    w = np.asarray(weights, dtype=np.float32)
    # [cout, cin, k] -> [cin, k, cout] so tap k is a contiguous lhsT slice
    wT = np.ascontiguousarray(w.transpose(1, 2, 0)).reshape(CIN, KK * KK * COUT)
    xpad = np.zeros((B, CIN, PH, PW), np.float32)
    xpad[:, :, 1 : H + 1, 1 : W + 1] = x
    wT = wT.astype(ml_dtypes.bfloat16)
    xpad = xpad.reshape(B, CIN, XLEN).astype(ml_dtypes.bfloat16)

    nc = _get_nc()
    xr = xpad.reshape(NCORES, BPC, CIN, XLEN)
    in_maps = [{"x": np.ascontiguousarray(xr[c]), "w": wT} for c in range(NCORES)]

    res = bass_utils.run_bass_kernel_spmd(
        nc, in_maps, core_ids=list(range(NCORES)), trace=TRACE
    )
    LAST_EXEC_NS = res.exec_time_ns

    arr = np.stack([res.results[c]["o"] for c in range(NCORES)])  # [8, 2, 128, 4096]
    # out[cout*B + b] = conv[b, cout], with b = core*BPC + lb
    arr = arr.transpose(2, 0, 1, 3).reshape(COUT, B, H, W)
    return np.ascontiguousarray(arr.reshape(COUT * B, H, W))


# revision 16
# speedup vs baseline: 1.0643x; 1.0643x over previous
"""Trainium2 Bass kernel for nn_CustomConv2d: 3x3 conv, B=16, Cin=Cout=128, H=W=64.

Strategy (v2, bf16):
  - Data-parallel over batch: 8 NeuronCores x 2 images each; the (128,128,9)
    weight is replicated (host pre-transposes it to [cin, k, cout] so tap k is
    a contiguous [cin, cout] stationary-operand slice).
  - Per image the feature map lives in SBUF as a 66x66 zero-padded plane
    (host-prepadded, so every DMA is fully contiguous):
      row r in [-1,64], col c in [-1,64] at offset (r+1)*66 + (c+1).
  - Conv = 9 accumulating PE matmuls per 8-row output block (contraction over
    Cin=128 on the partition dim).  Tap (dy,dx) reads the 2D window
    [[66,8],[1,64]] at offset (y0+dy)*66 + dx; the padding zeros make every
    tap exact, so there is no edge fixup of any kind.
  - bf16 operands (RNE-rounded on host): same 1 cycle/row PE speed as fp32r,
    but half the DMA bytes and a ~2x faster LDWEIGHTS, which is what gates
    the back-to-back matmul cadence on the Tensor queue.
  - Minimal 2-matmul PE warm-up (bf16 data lands early enough that real
    matmuls carry the HAM clock ramp themselves), few DMA triggers (6 in /
    10 out, stores batched 2 blocks each) to cut per-instruction queue cost
    and the Tile framework's per-event teardown chatter at kernel exit.
"""

import numpy as np
import ml_dtypes

import concourse.bass as bass  # noqa: F401  (registers bass types)
import concourse.tile as tile
import concourse.mybir as mybir
from concourse import bacc, bass_utils

F32 = mybir.dt.float32
BF16 = mybir.dt.bfloat16

B, CIN, COUT, KK, H, W = 16, 128, 128, 3, 64, 64
NCORES = 8
BPC = B // NCORES  # batches per core
HW = H * W         # 4096
PW = W + 2         # padded row length (66)
PH = H + 2         # padded rows (66)
XLEN = PH * PW     # 4356
ROWBLK = 8         # output rows per PSUM block (8*64=512 = one fp32 PSUM bank)
NBLK = H // ROWBLK

TRACE = False      # set True to capture an NTFF profile (fills LAST_EXEC_NS)
LAST_EXEC_NS = None

_CACHE = {}


def _build():
    nc = bacc.Bacc("TRN2", target_bir_lowering=False, debug=False, num_devices=NCORES)
    x_d = nc.dram_tensor("x", [BPC, CIN, XLEN], BF16, kind="ExternalInput").ap()
    w_d = nc.dram_tensor("w", [CIN, KK * KK * COUT], BF16, kind="ExternalInput").ap()
    o_d = nc.dram_tensor("o", [BPC, COUT, HW], F32, kind="ExternalOutput").ap()

    with tile.TileContext(nc) as tc:
        with (
            tc.tile_pool(name="wt", bufs=1) as wtp,
            tc.tile_pool(name="xin", bufs=2) as xp,
            tc.tile_pool(name="ps", bufs=4, space="PSUM") as pp,
            tc.tile_pool(name="ot", bufs=3) as op,
            tc.tile_pool(name="warm", bufs=1) as wmp,
            tc.tile_pool(name="warmps", bufs=1, space="PSUM") as wpp,
        ):
            # Short PE warm-up: just enough array activity to cover the first
            # input DMA's latency; the real matmuls continue the HAM clock
            # ramp (full speed arrives ~3.4us after sustained activity starts
            # either way).
            wz = wmp.tile([CIN, 4 * COUT], BF16)
            nc.vector.memset(wz[:], 0.0)
            wps = wpp.tile([COUT, 4 * COUT], F32)
            # the compiler always dead-code-eliminates the LAST warmup matmul
            # (unread PSUM write), so emit one extra: 6 emitted -> 5 on HW,
            # which bridges the PE from ~7.7us to the ~10.2us data-ready point
            # with no idle gap (any gap stretches the HAM clock ramp).
            NWARM = 6
            for i in range(NWARM):
                nc.tensor.matmul(
                    wps[:],
                    wz[:, (i % 4) * COUT : (i % 4 + 1) * COUT],
                    wz[:],
                    start=(i == 0),
                    stop=(i == NWARM - 1),
                )

            # All input loads on the SYNC ring only, in strict consumption
            # order.  (Both dual-ring variants regressed: the scalar ring is
            # handicapped by its ACT_TABLE_LOAD and loses DMA-backend
            # arbitration to the sync ring's queued bulk, starving critical
            # weight taps for microseconds.)
            # Block yb touches padded rows [8*yb, 8*yb+9].
            wt = wtp.tile([CIN, KK * KK * COUT], BF16)
            xins = []
            for lb in range(BPC):
                xin = xp.tile([CIN, XLEN], BF16, tag="xin")
                xins.append(xin)
            nc.sync.dma_start(wt[:, :COUT], w_d[:, :COUT])
            nc.sync.dma_start(xins[0][:, : PW * 10], x_d[0][:, : PW * 10])
            nc.sync.dma_start(wt[:, COUT : 3 * COUT], w_d[:, COUT : 3 * COUT])
            nc.sync.dma_start(wt[:, 3 * COUT :], w_d[:, 3 * COUT :])
            for r0, r1 in [(10, 38), (38, PH)]:
                nc.sync.dma_start(
                    xins[0][:, PW * r0 : PW * r1], x_d[0][:, PW * r0 : PW * r1]
                )
            for r0, r1 in [(0, 34), (34, PH)]:
                nc.sync.dma_start(
                    xins[1][:, PW * r0 : PW * r1], x_d[1][:, PW * r0 : PW * r1]
                )

            for lb in range(BPC):
                xrf = xins[lb][:].rearrange("p (r c) -> p r c", c=PW)  # [128,66,66]
                ot2 = None
                for yb in range(NBLK):
                    y0 = yb * ROWBLK
                    ps = pp.tile([COUT, ROWBLK * W], F32)
                    for t in range(KK * KK):
                        dy, dx = divmod(t, KK)
                        nc.tensor.matmul(
                            ps[:],
                            wt[:, t * COUT : (t + 1) * COUT],
                            xrf[:, y0 + dy : y0 + dy + ROWBLK, dx : dx + W],
                            start=(t == 0),
                            stop=(t == KK * KK - 1),
                        )
                    BLK = ROWBLK * W  # 512
                    if lb == BPC - 1 and yb == NBLK - 1:
                        # final block: halves copied by two engines (vector +
                        # scalar/ACT) and stored via two rings so the copy/
                        # store drain finishes sooner.  GPSIMD cannot read
                        # PSUM, so the second copy uses the ACT engine.
                        oths = []
                        for h_, cp in enumerate(
                            (nc.vector.tensor_copy, nc.scalar.copy)
                        ):
                            sl = slice(h_ * BLK // 2, (h_ + 1) * BLK // 2)
                            oth = op.tile([COUT, BLK // 2], F32, tag=f"oth{h_}")
                            cp(oth[:], ps[:, sl])
                            oths.append(oth)
                        for h_, deng in enumerate((nc.scalar, nc.sync)):
                            nc_lo = W * y0 + h_ * BLK // 2
                            deng.dma_start(
                                o_d[lb][:, nc_lo : nc_lo + BLK // 2], oths[h_][:]
                            )
                    elif lb == BPC - 1 and yb == NBLK - 2:
                        # penultimate block: its own small store
                        oth = op.tile([COUT, BLK], F32)
                        nc.vector.tensor_copy(oth[:], ps[:])
                        nc.scalar.dma_start(o_d[lb][:, W * y0 : W * y0 + BLK], oth[:])
                    else:
                        # batch 2 row-blocks per store
                        pair = yb % 2
                        if pair == 0:
                            ot2 = op.tile([COUT, 2 * BLK], F32)
                        nc.vector.tensor_copy(ot2[:, pair * BLK : (pair + 1) * BLK], ps[:])
                        if pair == 1:
                            nc.scalar.dma_start(
                                o_d[lb][:, W * y0 - BLK : W * y0 + BLK], ot2[:]
                            )
    nc.compile()
    return nc


def _get_nc():
    if "nc" not in _CACHE:
        _CACHE["nc"] = _build()
    return _CACHE["nc"]


def kernel(x, weights):
    """x: [16,128,64,64] f32; weights: [128,128,9] f32 -> [2048,64,64] f32."""
    global LAST_EXEC_NS
    x = np.asarray(x, dtype=np.float32)
    w = np.asarray(weights, dtype=np.float32)
    # [cout, cin, k] -> [cin, k, cout] so tap k is a contiguous lhsT slice
    wT = np.ascontiguousarray(w.transpose(1, 2, 0)).reshape(CIN, KK * KK * COUT)
    xpad = np.zeros((B, CIN, PH, PW), np.float32)
    xpad[:, :, 1 : H + 1, 1 : W + 1] = x
    wT = wT.astype(ml_dtypes.bfloat16)
    xpad = xpad.reshape(B, CIN, XLEN).astype(ml_dtypes.bfloat16)

    nc = _get_nc()
    xr = xpad.reshape(NCORES, BPC, CIN, XLEN)
    in_maps = [{"x": np.ascontiguousarray(xr[c]), "w": wT} for c in range(NCORES)]

    res = bass_utils.run_bass_kernel_spmd(
        nc, in_maps, core_ids=list(range(NCORES)), trace=TRACE
    )
    LAST_EXEC_NS = res.exec_time_ns

    arr = np.stack([res.results[c]["o"] for c in range(NCORES)])  # [8, 2, 128, 4096]
    # out[cout*B + b] = conv[b, cout], with b = core*BPC + lb
    arr = arr.transpose(2, 0, 1, 3).reshape(COUT, B, H, W)
    return np.ascontiguousarray(arr.reshape(COUT * B, H, W))


# revision 19
# speedup vs baseline: 1.0793x; 1.0141x over previous
"""Trainium2 Bass kernel for nn_CustomConv2d: 3x3 conv, B=16, Cin=Cout=128, H=W=64.

Strategy (v2, bf16):
  - Data-parallel over batch: 8 NeuronCores x 2 images each; the (128,128,9)
    weight is replicated (host pre-transposes it to [cin, k, cout] so tap k is
    a contiguous [cin, cout] stationary-operand slice).
  - Per image the feature map lives in SBUF as a 66x66 zero-padded plane
    (host-prepadded, so every DMA is fully contiguous):
      row r in [-1,64], col c in [-1,64] at offset (r+1)*66 + (c+1).
  - Conv = 9 accumulating PE matmuls per 8-row output block (contraction over
    Cin=128 on the partition dim).  Tap (dy,dx) reads the 2D window
    [[66,8],[1,64]] at offset (y0+dy)*66 + dx; the padding zeros make every
    tap exact, so there is no edge fixup of any kind.
  - bf16 operands (RNE-rounded on host): same 1 cycle/row PE speed as fp32r,
    but half the DMA bytes and a ~2x faster LDWEIGHTS, which is what gates
    the back-to-back matmul cadence on the Tensor queue.
  - Minimal 2-matmul PE warm-up (bf16 data lands early enough that real
    matmuls carry the HAM clock ramp themselves), few DMA triggers (6 in /
    10 out, stores batched 2 blocks each) to cut per-instruction queue cost
    and the Tile framework's per-event teardown chatter at kernel exit.
"""

import numpy as np
import ml_dtypes

import concourse.bass as bass  # noqa: F401  (registers bass types)
import concourse.tile as tile
import concourse.mybir as mybir
from concourse import bacc, bass_utils

F32 = mybir.dt.float32
BF16 = mybir.dt.bfloat16

B, CIN, COUT, KK, H, W = 16, 128, 128, 3, 64, 64
NCORES = 8
BPC = B // NCORES  # batches per core
HW = H * W         # 4096
PW = W + 2         # padded row length (66)
PH = H + 2         # padded rows (66)
XLEN = PH * PW     # 4356
ROWBLK = 8         # output rows per PSUM block (8*64=512 = one fp32 PSUM bank)
NBLK = H // ROWBLK

TRACE = False      # set True to capture an NTFF profile (fills LAST_EXEC_NS)
LAST_EXEC_NS = None

_CACHE = {}


def _build():
    nc = bacc.Bacc("TRN2", target_bir_lowering=False, debug=False, num_devices=NCORES)
    x_d = nc.dram_tensor("x", [BPC, CIN, XLEN], BF16, kind="ExternalInput").ap()
    w_d = nc.dram_tensor("w", [CIN, KK * KK * COUT], BF16, kind="ExternalInput").ap()
    o_d = nc.dram_tensor("o", [BPC, COUT, HW], F32, kind="ExternalOutput").ap()

    with tile.TileContext(nc) as tc:
        with (
            tc.tile_pool(name="wt", bufs=1) as wtp,
            tc.tile_pool(name="xin", bufs=2) as xp,
            tc.tile_pool(name="ps", bufs=4, space="PSUM") as pp,
            tc.tile_pool(name="ot", bufs=3) as op,
            tc.tile_pool(name="warm", bufs=1) as wmp,
            tc.tile_pool(name="warmps", bufs=1, space="PSUM") as wpp,
        ):
            # Short PE warm-up: just enough array activity to cover the first
            # input DMA's latency; the real matmuls continue the HAM clock
            # ramp (full speed arrives ~3.4us after sustained activity starts
            # either way).
            wz = wmp.tile([CIN, 4 * COUT], BF16)
            nc.vector.memset(wz[:], 0.0)
            wps = wpp.tile([COUT, 4 * COUT], F32)
            # the compiler always dead-code-eliminates the LAST warmup matmul
            # (unread PSUM write), so emit one extra: 6 emitted -> 5 on HW,
            # which bridges the PE from ~7.7us to the ~10.2us data-ready point
            # with no idle gap (any gap stretches the HAM clock ramp).
            NWARM = 6
            for i in range(NWARM):
                nc.tensor.matmul(
                    wps[:],
                    wz[:, (i % 4) * COUT : (i % 4 + 1) * COUT],
                    wz[:],
                    start=(i == 0),
                    stop=(i == NWARM - 1),
                )

            # All input loads on the SYNC ring only, in strict consumption
            # order.  (Both dual-ring variants regressed: the scalar ring is
            # handicapped by its ACT_TABLE_LOAD and loses DMA-backend
            # arbitration to the sync ring's queued bulk, starving critical
            # weight taps for microseconds.)
            # Block yb touches padded rows [8*yb, 8*yb+9].
            wt = wtp.tile([CIN, KK * KK * COUT], BF16)
            xins = []
            for lb in range(BPC):
                xin = xp.tile([CIN, XLEN], BF16, tag="xin")
                xins.append(xin)
            nc.sync.dma_start(wt[:, :COUT], w_d[:, :COUT])
            nc.sync.dma_start(xins[0][:, : PW * 10], x_d[0][:, : PW * 10])
            nc.sync.dma_start(wt[:, COUT : 3 * COUT], w_d[:, COUT : 3 * COUT])
            nc.sync.dma_start(wt[:, 3 * COUT :], w_d[:, 3 * COUT :])
            for r0, r1 in [(10, 24), (24, 38), (38, PH)]:
                nc.sync.dma_start(
                    xins[0][:, PW * r0 : PW * r1], x_d[0][:, PW * r0 : PW * r1]
                )
            for r0, r1 in [(0, 34), (34, PH)]:
                nc.sync.dma_start(
                    xins[1][:, PW * r0 : PW * r1], x_d[1][:, PW * r0 : PW * r1]
                )

            for lb in range(BPC):
                xrf = xins[lb][:].rearrange("p (r c) -> p r c", c=PW)  # [128,66,66]
                ot2 = None
                for yb in range(NBLK):
                    y0 = yb * ROWBLK
                    BLK = ROWBLK * W  # 512
                    if lb == BPC - 1 and yb == NBLK - 1:
                        # final block: two 4-row PSUM groups so the drain
                        # chain after the very last matmul is half-sized and
                        # the first half's copy/store overlaps the second
                        # half's matmuls.  GPSIMD cannot read PSUM, so the
                        # first copy uses the ACT engine.
                        for h_, (cp, deng) in enumerate(
                            (
                                (nc.scalar.copy, nc.scalar),
                                (nc.vector.tensor_copy, nc.sync),
                            )
                        ):
                            psh = pp.tile([COUT, BLK // 2], F32, tag="ps")
                            for t in range(KK * KK):
                                dy, dx = divmod(t, KK)
                                nc.tensor.matmul(
                                    psh[:],
                                    wt[:, t * COUT : (t + 1) * COUT],
                                    xrf[
                                        :,
                                        y0 + 4 * h_ + dy : y0 + 4 * h_ + dy + 4,
                                        dx : dx + W,
                                    ],
                                    start=(t == 0),
                                    stop=(t == KK * KK - 1),
                                )
                            oth = op.tile([COUT, BLK // 2], F32, tag=f"oth{h_}")
                            cp(oth[:], psh[:])
                            nc_lo = W * y0 + h_ * BLK // 2
                            deng.dma_start(
                                o_d[lb][:, nc_lo : nc_lo + BLK // 2], oth[:]
                            )
                        continue
                    ps = pp.tile([COUT, ROWBLK * W], F32)
                    for t in range(KK * KK):
                        dy, dx = divmod(t, KK)
                        nc.tensor.matmul(
                            ps[:],
                            wt[:, t * COUT : (t + 1) * COUT],
                            xrf[:, y0 + dy : y0 + dy + ROWBLK, dx : dx + W],
                            start=(t == 0),
                            stop=(t == KK * KK - 1),
                        )
                    if lb == BPC - 1 and yb == NBLK - 2:
                        # penultimate block: its own small store
                        oth = op.tile([COUT, BLK], F32)
                        nc.vector.tensor_copy(oth[:], ps[:])
                        nc.scalar.dma_start(o_d[lb][:, W * y0 : W * y0 + BLK], oth[:])
                    else:
                        # batch 2 row-blocks per store
                        pair = yb % 2
                        if pair == 0:
                            ot2 = op.tile([COUT, 2 * BLK], F32)
                        nc.vector.tensor_copy(ot2[:, pair * BLK : (pair + 1) * BLK], ps[:])
                        if pair == 1:
                            nc.scalar.dma_start(
                                o_d[lb][:, W * y0 - BLK : W * y0 + BLK], ot2[:]
                            )
    nc.compile()
    return nc


def _get_nc():
    if "nc" not in _CACHE:
        _CACHE["nc"] = _build()
    return _CACHE["nc"]


def kernel(x, weights):
    """x: [16,128,64,64] f32; weights: [128,128,9] f32 -> [2048,64,64] f32."""
    global LAST_EXEC_NS
    x = np.asarray(x, dtype=np.float32)
    w = np.asarray(weights, dtype=np.float32)
    # [cout, cin, k] -> [cin, k, cout] so tap k is a contiguous lhsT slice
    wT = np.ascontiguousarray(w.transpose(1, 2, 0)).reshape(CIN, KK * KK * COUT)
    xpad = np.zeros((B, CIN, PH, PW), np.float32)
    xpad[:, :, 1 : H + 1, 1 : W + 1] = x
    wT = wT.astype(ml_dtypes.bfloat16)
    xpad = xpad.reshape(B, CIN, XLEN).astype(ml_dtypes.bfloat16)

    nc = _get_nc()
    xr = xpad.reshape(NCORES, BPC, CIN, XLEN)
    in_maps = [{"x": np.ascontiguousarray(xr[c]), "w": wT} for c in range(NCORES)]

    res = bass_utils.run_bass_kernel_spmd(
        nc, in_maps, core_ids=list(range(NCORES)), trace=TRACE
    )
    LAST_EXEC_NS = res.exec_time_ns

    arr = np.stack([res.results[c]["o"] for c in range(NCORES)])  # [8, 2, 128, 4096]
    # out[cout*B + b] = conv[b, cout], with b = core*BPC + lb
    arr = arr.transpose(2, 0, 1, 3).reshape(COUT, B, H, W)
    return np.ascontiguousarray(arr.reshape(COUT * B, H, W))


# revision 20
# speedup vs baseline: 1.0923x; 1.0121x over previous
"""Trainium2 Bass kernel for nn_CustomConv2d: 3x3 conv, B=16, Cin=Cout=128, H=W=64.

Strategy (v2, bf16):
  - Data-parallel over batch: 8 NeuronCores x 2 images each; the (128,128,9)
    weight is replicated (host pre-transposes it to [cin, k, cout] so tap k is
    a contiguous [cin, cout] stationary-operand slice).
  - Per image the feature map lives in SBUF as a 66x66 zero-padded plane
    (host-prepadded, so every DMA is fully contiguous):
      row r in [-1,64], col c in [-1,64] at offset (r+1)*66 + (c+1).
  - Conv = 9 accumulating PE matmuls per 8-row output block (contraction over
    Cin=128 on the partition dim).  Tap (dy,dx) reads the 2D window
    [[66,8],[1,64]] at offset (y0+dy)*66 + dx; the padding zeros make every
    tap exact, so there is no edge fixup of any kind.
  - bf16 operands (RNE-rounded on host): same 1 cycle/row PE speed as fp32r,
    but half the DMA bytes and a ~2x faster LDWEIGHTS, which is what gates
    the back-to-back matmul cadence on the Tensor queue (218 vs 240 ns).
  - 5 warm-up matmuls (6 emitted; the compiler always DCEs the last one)
    bridge the PE gap-free from the end of the fixed ~7us framework preamble
    to the ~10.1us point where the critical first DMA bytes (w tap 0 + the
    first 10 padded input rows) have landed - any PE idle gap stretches the
    HAM clock ramp (full clock ~4.4us after sustained array activity starts,
    and the DMA clock ramps with it).
  - All input DMAs ride the sync HWDGE ring in strict consumption order
    (dual-ring splits starve: the scalar ring loses backend arbitration).
    Stores ride the scalar ring, batched 2 row-blocks each; the final block
    runs as two 4-row PSUM groups whose copy/store drain overlaps the last
    matmuls, finishing on both rings ~1.1us after the last matmul.
"""

import numpy as np
import ml_dtypes

import concourse.bass as bass  # noqa: F401  (registers bass types)
import concourse.tile as tile
import concourse.mybir as mybir
from concourse import bacc, bass_utils

F32 = mybir.dt.float32
BF16 = mybir.dt.bfloat16

B, CIN, COUT, KK, H, W = 16, 128, 128, 3, 64, 64
NCORES = 8
BPC = B // NCORES  # batches per core
HW = H * W         # 4096
PW = W + 2         # padded row length (66)
PH = H + 2         # padded rows (66)
XLEN = PH * PW     # 4356
ROWBLK = 8         # output rows per PSUM block (8*64=512 = one fp32 PSUM bank)
NBLK = H // ROWBLK

TRACE = False      # set True to capture an NTFF profile (fills LAST_EXEC_NS)
LAST_EXEC_NS = None

_CACHE = {}


def _build():
    nc = bacc.Bacc("TRN2", target_bir_lowering=False, debug=False, num_devices=NCORES)
    x_d = nc.dram_tensor("x", [BPC, CIN, XLEN], BF16, kind="ExternalInput").ap()
    w_d = nc.dram_tensor("w", [CIN, KK * KK * COUT], BF16, kind="ExternalInput").ap()
    o_d = nc.dram_tensor("o", [BPC, COUT, HW], F32, kind="ExternalOutput").ap()

    with tile.TileContext(nc) as tc:
        with (
            tc.tile_pool(name="wt", bufs=1) as wtp,
            tc.tile_pool(name="xin", bufs=2) as xp,
            tc.tile_pool(name="ps", bufs=4, space="PSUM") as pp,
            tc.tile_pool(name="ot", bufs=3) as op,
            tc.tile_pool(name="warm", bufs=1) as wmp,
            tc.tile_pool(name="warmps", bufs=1, space="PSUM") as wpp,
        ):
            # Short PE warm-up: just enough array activity to cover the first
            # input DMA's latency; the real matmuls continue the HAM clock
            # ramp (full speed arrives ~3.4us after sustained activity starts
            # either way).
            wz = wmp.tile([CIN, 4 * COUT], BF16)
            nc.vector.memset(wz[:], 0.0)
            wps = wpp.tile([COUT, 4 * COUT], F32)
            # the compiler always dead-code-eliminates the LAST warmup matmul
            # (unread PSUM write), so emit one extra: 6 emitted -> 5 on HW,
            # which bridges the PE from ~7.7us to the ~10.2us data-ready point
            # with no idle gap (any gap stretches the HAM clock ramp).
            NWARM = 6
            for i in range(NWARM):
                nc.tensor.matmul(
                    wps[:],
                    wz[:, (i % 4) * COUT : (i % 4 + 1) * COUT],
                    wz[:],
                    start=(i == 0),
                    stop=(i == NWARM - 1),
                )

            # All input loads on the SYNC ring only, in strict consumption
            # order.  (Both dual-ring variants regressed: the scalar ring is
            # handicapped by its ACT_TABLE_LOAD and loses DMA-backend
            # arbitration to the sync ring's queued bulk, starving critical
            # weight taps for microseconds.)
            # Block yb touches padded rows [8*yb, 8*yb+9].
            wt = wtp.tile([CIN, KK * KK * COUT], BF16)
            xins = []
            for lb in range(BPC):
                xin = xp.tile([CIN, XLEN], BF16, tag="xin")
                xins.append(xin)
            nc.sync.dma_start(wt[:, :COUT], w_d[:, :COUT])
            nc.sync.dma_start(xins[0][:, : PW * 10], x_d[0][:, : PW * 10])
            nc.sync.dma_start(wt[:, COUT : 3 * COUT], w_d[:, COUT : 3 * COUT])
            nc.sync.dma_start(wt[:, 3 * COUT :], w_d[:, 3 * COUT :])
            for r0, r1 in [(10, 24), (24, 38), (38, PH)]:
                nc.sync.dma_start(
                    xins[0][:, PW * r0 : PW * r1], x_d[0][:, PW * r0 : PW * r1]
                )
            for r0, r1 in [(0, 34), (34, PH)]:
                nc.sync.dma_start(
                    xins[1][:, PW * r0 : PW * r1], x_d[1][:, PW * r0 : PW * r1]
                )

            for lb in range(BPC):
                xrf = xins[lb][:].rearrange("p (r c) -> p r c", c=PW)  # [128,66,66]
                ot2 = None
                for yb in range(NBLK):
                    y0 = yb * ROWBLK
                    BLK = ROWBLK * W  # 512
                    if lb == BPC - 1 and yb == NBLK - 1:
                        # final block: two 4-row PSUM groups so the drain
                        # chain after the very last matmul is half-sized and
                        # the first half's copy/store overlaps the second
                        # half's matmuls.  GPSIMD cannot read PSUM, so the
                        # first copy uses the ACT engine.
                        for h_, (cp, deng) in enumerate(
                            (
                                (nc.scalar.copy, nc.scalar),
                                (nc.vector.tensor_copy, nc.sync),
                            )
                        ):
                            psh = pp.tile([COUT, BLK // 2], F32, tag="ps")
                            for t in range(KK * KK):
                                dy, dx = divmod(t, KK)
                                nc.tensor.matmul(
                                    psh[:],
                                    wt[:, t * COUT : (t + 1) * COUT],
                                    xrf[
                                        :,
                                        y0 + 4 * h_ + dy : y0 + 4 * h_ + dy + 4,
                                        dx : dx + W,
                                    ],
                                    start=(t == 0),
                                    stop=(t == KK * KK - 1),
                                )
                            oth = op.tile([COUT, BLK // 2], F32, tag=f"oth{h_}")
                            cp(oth[:], psh[:])
                            nc_lo = W * y0 + h_ * BLK // 2
                            deng.dma_start(
                                o_d[lb][:, nc_lo : nc_lo + BLK // 2], oth[:]
                            )
                        continue
                    ps = pp.tile([COUT, ROWBLK * W], F32)
                    for t in range(KK * KK):
                        dy, dx = divmod(t, KK)
                        nc.tensor.matmul(
                            ps[:],
                            wt[:, t * COUT : (t + 1) * COUT],
                            xrf[:, y0 + dy : y0 + dy + ROWBLK, dx : dx + W],
                            start=(t == 0),
                            stop=(t == KK * KK - 1),
                        )
                    if lb == BPC - 1 and yb == NBLK - 2:
                        # penultimate block: its own small store
                        oth = op.tile([COUT, BLK], F32)
                        nc.vector.tensor_copy(oth[:], ps[:])
                        nc.scalar.dma_start(o_d[lb][:, W * y0 : W * y0 + BLK], oth[:])
                    else:
                        # batch 2 row-blocks per store
                        pair = yb % 2
                        if pair == 0:
                            ot2 = op.tile([COUT, 2 * BLK], F32)
                        nc.vector.tensor_copy(ot2[:, pair * BLK : (pair + 1) * BLK], ps[:])
                        if pair == 1:
                            nc.scalar.dma_start(
                                o_d[lb][:, W * y0 - BLK : W * y0 + BLK], ot2[:]
                            )
    nc.compile()
    return nc


def _get_nc():
    if "nc" not in _CACHE:
        _CACHE["nc"] = _build()
    return _CACHE["nc"]


def kernel(x, weights):
    """x: [16,128,64,64] f32; weights: [128,128,9] f32 -> [2048,64,64] f32."""
    global LAST_EXEC_NS
    x = np.asarray(x, dtype=np.float32)
    w = np.asarray(weights, dtype=np.float32)
    # [cout, cin, k] -> [cin, k, cout] so tap k is a contiguous lhsT slice
    wT = np.ascontiguousarray(w.transpose(1, 2, 0)).reshape(CIN, KK * KK * COUT)
    xpad = np.zeros((B, CIN, PH, PW), np.float32)
    xpad[:, :, 1 : H + 1, 1 : W + 1] = x
    wT = wT.astype(ml_dtypes.bfloat16)
    xpad = xpad.reshape(B, CIN, XLEN).astype(ml_dtypes.bfloat16)

    nc = _get_nc()
    xr = xpad.reshape(NCORES, BPC, CIN, XLEN)
    in_maps = [{"x": np.ascontiguousarray(xr[c]), "w": wT} for c in range(NCORES)]

    res = bass_utils.run_bass_kernel_spmd(
        nc, in_maps, core_ids=list(range(NCORES)), trace=TRACE
    )
    LAST_EXEC_NS = res.exec_time_ns

    arr = np.stack([res.results[c]["o"] for c in range(NCORES)])  # [8, 2, 128, 4096]
    # out[cout*B + b] = conv[b, cout], with b = core*BPC + lb
    arr = arr.transpose(2, 0, 1, 3).reshape(COUT, B, H, W)
    return np.ascontiguousarray(arr.reshape(COUT * B, H, W))
